# revision 72
# baseline (speedup 1.0000x reference)
"""Causal attention kernel for Trainium2, 8 NeuronCores.

Problem: x[4,4096,768] f32; Wq/Wk/Wv [768,64] f32.
  q,k,v = x@W*; S = q@k.T (causal); out = softmax(S/8)@v -> [4,4096,64] f32.

Strategy: chained query-range shards, data-parallel over batch. The 4096
query rows split into contiguous ranges (SHARDS); launch i runs range i
for all 4 batches (one core per batch, alternating core groups 0-3/4-7).
Launches run back-to-back; each is an independently profiled program.

Per-shard device algorithm (q rows [a,b), keys [0,b)):
  - reads kT [64, a] (e-major) and vx [128, a/128*65] (token-major v with
    a ones column) for keys below its range from HBM -- written by the
    earlier shards -- and projects q/k/v only for its own [a,b) tokens.
  - scores transposed per (key-tile 128 x q-chunk) block on PE:
    ST[j,i] = sum_e kT[e,j] qT[e,i], causally trimmed.
  - P = exp(ST/8) via ScalarE into bf16 (no max subtraction: |S/8| small);
    diagonal 128x128 blocks masked by a triangular 0/1 mask on DVE.
  - attention output accumulated TRANSPOSED, one PSUM tile per q-block:
    o[q, 0:65] += P[k, q-block].T @ vx[k, 0:65]; the stationary operand is
    the P block, the 65-wide moving operand makes the PV matmuls cheap,
    and the ones column of vx accumulates the softmax denominators.
  - o[r, 65] f32 is DMA'd out token-major; the host does out = o[:,:64]/o[:,64:]
    (normalization only; no transposes).
"""

import numpy as np
import ml_dtypes

import concourse.bass as bass  # noqa: F401  (bacc pulls it in)
import concourse.bacc as bacc
import concourse.mybir as mybir
import concourse.tile as tile
from concourse.bass_utils import run_bass_kernel_spmd

B, N, D_IN, D_OUT = 4, 4096, 768, 64
NDC = D_IN // 128  # contraction chunks
BF16 = mybir.dt.bfloat16
F32 = mybir.dt.float32
SCALE = 1.0 / 8.0  # 1/sqrt(64)

# q-range boundaries of the shard chain (each a multiple of 128).
SHARDS = [0, 1024, 1664, 2176, 2560, 2944, 3328, 3584, 3840, 4096]


RAMP = (128, 128, 256)


def _chunks_for(a, b, ramp):
    """q-chunk widths; small leading chunks let ScalarE start while the
    xT stream is still arriving."""
    out = []
    c0 = a
    if ramp:
        for w in RAMP:
            if c0 + w <= b:
                out.append((c0, w))
                c0 += w
    while c0 < b:
        w = min(512, b - c0)
        if w == 384:
            # widths must be powers of two: the score-strip sections are
            # packed at Nc stride, and a matmul output must not cross a
            # PSUM bank boundary (2KB); 384-wide sections would.
            w = 256
        out.append((c0, w))
        c0 += w
    return out


def _piece_bounds(a, b, ramp):
    """Token-piece boundaries for the xT load + projection groups."""
    bounds = [a]
    if ramp:
        for w in RAMP:
            if bounds[-1] + w <= b:
                bounds.append(bounds[-1] + w)
    while bounds[-1] < b:
        bounds.append(min(bounds[-1] + 512, b))
    return bounds


def build_shard(a, b):
    """Build the Bass program for q rows [a, b) (keys [0, b))."""
    r = b - a
    H = a // 128  # handoff key tiles
    TT = b // 128  # total key tiles
    nto = r // 128  # own key tiles
    KGRP = 2  # key tiles per score group (x2-buffered = 4 PSUM banks)
    ramp = True

    nc = bacc.Bacc("TRN2", target_bir_lowering=False, debug=False)

    xT_d = nc.dram_tensor("xT", [D_IN, r], BF16, kind="ExternalInput")
    const_d = nc.dram_tensor(
        "consts", [128, NDC * 192 + 192], BF16, kind="ExternalInput"
    )
    if a:
        kT_in_d = nc.dram_tensor("kT_in", [64, a], BF16, kind="ExternalInput")
        vx_in_d = nc.dram_tensor("vx_in", [128, H * 65], BF16, kind="ExternalInput")
    kT_out_d = nc.dram_tensor("kT_out", [64, r], BF16, kind="ExternalOutput")
    vx_out_d = nc.dram_tensor("vx_out", [128, nto * 65], BF16, kind="ExternalOutput")
    o_d = nc.dram_tensor("o", [r, 65], F32, kind="ExternalOutput")

    from contextlib import ExitStack

    with tile.TileContext(nc) as tc, ExitStack() as stk:
        cpool = stk.enter_context(tc.tile_pool(name="const", bufs=1))
        xpool = stk.enter_context(tc.tile_pool(name="xt", bufs=1))
        jpool = stk.enter_context(tc.tile_pool(name="proj", bufs=1))
        ppool = stk.enter_context(tc.tile_pool(name="pp", bufs=4))
        fpool = stk.enter_context(tc.tile_pool(name="fin", bufs=2))

        # ---- constants (one packed load: wqkv | mask | ident) ----
        const_sb = cpool.tile([128, NDC * 192 + 192], BF16, tag="const")
        w3 = const_sb[:, 0 : NDC * 192].rearrange("p (c j) -> p c j", j=192)
        mask_sb = const_sb[:, NDC * 192 : NDC * 192 + 128]
        ident_sb = const_sb[0:64, NDC * 192 + 128 : NDC * 192 + 192]
        zbias = cpool.tile([128, 1], F32, tag="zbias")
        nc.vector.memset(zbias[:, :], 0.0)
        zeros_sb = cpool.tile([128, 260], BF16, tag="zeros")
        nc.vector.memset(zeros_sb[:, :], 0.0)

        # ---- SBUF buffers ----
        xt_sb = xpool.tile([128, NDC * r], BF16, tag="xt")
        xt3 = xt_sb.rearrange("p (c n) -> p c n", n=r)
        xT3d = xT_d.ap().rearrange("(c p) n -> p c n", p=128)
        # kq band: rows 0:64 vT (own cols), rows 64:128 kT (cols 0:b) and
        # qT (cols b:b+r). kT and qT share base partition 64 for the
        # S-matmul.
        kq_sb = jpool.tile([128, b + r], BF16, tag="kq")
        vx_sb = jpool.tile([128, TT * 65], BF16, tag="vx")
        vx3 = vx_sb.rearrange("p (t e) -> p t e", e=65)
        # softmax-denominator ones column for all own tiles, set once
        # (handoff tiles arrive from HBM with their ones already set)
        nc.vector.memset(vx3[:, H:TT, 64:65], 1.0)

        qT = kq_sb[64:128, b : b + r]

        def kT(t):
            return kq_sb[64:128, 128 * t : 128 * (t + 1)]

        # ---- input DMAs (interleaved so early pieces land first) ----
        xb = _piece_bounds(a, b, ramp)
        xt_pieces = list(zip(xb[:-1], xb[1:]))
        kv_pieces = []
        if a:
            kb = [0]
            while kb[-1] < a:
                kb.append(min(kb[-1] + (512 if len(kb) < 3 else 1024), a))
            kv_pieces = list(zip(kb[:-1], kb[1:]))

        # input loads: the HWDGE device is shared across SP/ACT queues and
        # holds ~630ns per DMA, so keep the critical-path loads (consts,
        # xT stream, first kT piece) on HWDGE queues; everything else goes
        # via the gpsimd SWDGE path, which generates descriptors on the
        # otherwise-idle Pool engine. The kT stream (128B/key) is kept
        # separate from the 3x-bigger vx payload so scores never starve.
        nc.sync.dma_start(const_sb[:, :], const_d.ap())
        for g0, g1 in xt_pieces:
            nc.sync.dma_start(xt3[:, :, g0 - a : g1 - a], xT3d[:, :, g0 - a : g1 - a])
        for i, (k0, k1) in enumerate(kv_pieces):
            eng = nc.scalar if i == 0 else nc.gpsimd
            eng.dma_start(kq_sb[64:128, k0:k1], kT_in_d.ap()[:, k0:k1])
        if a:
            vx3d = vx_in_d.ap().rearrange("p (t e) -> p t e", e=65)
            h2 = max(1, H // 2)
            nc.gpsimd.dma_start(vx3[:, 0:h2, :], vx3d[:, 0:h2, :])
            if h2 < H:
                nc.gpsimd.dma_start(vx3[:, h2:H, :], vx3d[:, h2:H, :])

        # ---- projections ----
        done = {"q": a, "kv": a}
        bset = sorted(set(xb))

        def _grp_end(g0):
            import bisect

            i = bisect.bisect_right(bset, g0)
            return bset[i] if i < len(bset) else b

        def emit_q_upto(tok, pool):
            while done["q"] < min(tok, b):
                g0 = done["q"]
                g = min(_grp_end(g0), b) - g0
                ps = pool.tile([128, 512], F32, tag="proj", name="ps")
                for dc in range(NDC):
                    nc.tensor.matmul(
                        ps[64:128, 0:g],
                        lhsT=w3[:, dc, 0:64],
                        rhs=xt3[:, dc, g0 - a : g0 - a + g],
                        start=(dc == 0),
                        stop=(dc == NDC - 1),
                        tile_position=(0, 64),
                    )
                nc.vector.tensor_copy(
                    kq_sb[64:128, b + g0 - a : b + g0 - a + g], ps[64:128, 0:g]
                )
                done["q"] = g0 + g

        def emit_vx(t0, t1, pool):
            # v -> token-major on the PE (cheap 64-row transpose matmuls;
            # keeps the chain off the DMA/HWDGE queues), then one DVE copy
            # into the vx slots. The ones column was memset up front.
            tp = pool.tile([128, 512], BF16, tag="proj", name="tp")
            for ti in range(t1 - t0):
                nc.tensor.transpose(
                    tp[:, 64 * ti : 64 * ti + 64],
                    kq_sb[0:64, 128 * (t0 + ti) : 128 * (t0 + ti + 1)],
                    ident_sb[:, :],
                )
            nc.vector.tensor_copy(
                vx3[:, t0:t1, 0:64],
                tp.rearrange("p (t e) -> p t e", e=64)[:, 0 : t1 - t0, :],
            )
            # handoff to later shards
            nc.gpsimd.dma_start(
                kT_out_d.ap()[:, 128 * t0 - a : 128 * t1 - a],
                kq_sb[64:128, 128 * t0 : 128 * t1],
            )
            nc.gpsimd.dma_start(
                vx_out_d.ap().rearrange("p (t e) -> p t e", e=65)[
                    :, t0 - H : t1 - H, :
                ],
                vx3[:, t0:t1, :],
            )

        def emit_kv_upto(tok, pool):
            while done["kv"] < min(tok, b):
                g0 = done["kv"]
                g = min(_grp_end(g0), b) - g0
                ps = pool.tile([128, 512], F32, tag="proj", name="ps")
                for dc in range(NDC):
                    nc.tensor.matmul(
                        ps[:, 0:g],
                        lhsT=w3[:, dc, 64:192],
                        rhs=xt3[:, dc, g0 - a : g0 - a + g],
                        start=(dc == 0),
                        stop=(dc == NDC - 1),
                    )
                nc.vector.tensor_copy(kq_sb[:, g0 : g0 + g], ps[:, 0:g])
                done["kv"] = g0 + g
                # defer the v-transpose one step so its dep (the kv copy)
                # is met by the time it reaches the in-order PE sequencer
                t0, t1 = g0 // 128, (g0 + g) // 128
                if t1 > t0:
                    if done.get("vxp") is not None:
                        done["vxp"]()
                    done["vxp"] = lambda t0=t0, t1=t1: emit_vx(t0, t1, pool)

        def flush_vx():
            if done.get("vxp") is not None:
                done["vxp"]()
                done["vxp"] = None

        # partial-tile leftovers are impossible: bounds are 128-aligned

        # ---- attention ----
        def attention(spool, opool, pool_for_proj):
            chunks = _chunks_for(a, b, ramp)
            # deferred closures (PV of an earlier group / o drains): emitted
            # one group late so their deps are met when they reach the PE
            # sequencer -- parked instructions fill the 4-deep wait queue and
            # stall everything behind them.
            pending = []

            def flush(keep=0):
                while len(pending) > keep:
                    pending.pop(0)()

            for ci, (qc0, Nc) in enumerate(chunks):
                last_chunk = ci == len(chunks) - 1
                nqb = Nc // 128
                T_c = (qc0 + Nc) // 128
                # narrow chunks pack more key tiles per PSUM slot / exp call
                kge = max(KGRP, (KGRP * 512) // Nc)
                if pool_for_proj is not None:
                    # hard guards (normally no-ops: the per-group proj steps
                    # below keep projections ahead of their consumers)
                    emit_q_upto(qc0 + Nc, pool_for_proj)
                    emit_kv_upto(T_c * 128, pool_for_proj)
                    flush_vx()

                def proj_step(qc0=qc0, Nc=Nc):
                    # advance up to one q and one kv projection group per
                    # attention group: spreads proj matmuls through the PE
                    # stream so they fill ACT-bound bubbles without parking
                    # the in-order sequencer on the proj PSUM pool. kv leads
                    # by a chunk so the vx chain stays ahead of its PV use.
                    if pool_for_proj is None:
                        return
                    if done["q"] < min(qc0 + Nc + 512, b):
                        emit_q_upto(done["q"] + 1, pool_for_proj)
                    if done["kv"] < min(qc0 + Nc + 1024, b):
                        emit_kv_upto(done["kv"] + 1, pool_for_proj)
                tiles = list(range(T_c))
                groups = [tiles[t0 : t0 + kge] for t0 in range(0, T_c, kge)]
                o_tile = opool.tile([128, 260], F32, tag="o", name="o_tile")
                # single full-width start for the whole o tile: a matmul's
                # start=True clears has_written for the entire PSUM bank, so
                # per-q-block chains must NOT each open their own group --
                # later starts would flip earlier chains' columns back to
                # overwrite mode and drop their first-tile contributions.
                nc.tensor.matmul(
                    o_tile[:, 0 : 65 * nqb],
                    lhsT=zeros_sb[:, 0:128],
                    rhs=zeros_sb[:, 0 : 65 * nqb],
                    start=True,
                    stop=False,
                    skip_group_check=True,
                )

                def emit_s(grp, qc0=qc0, Nc=Nc):
                    # all tiles of the group write cols [i0g, Nc): the ACT
                    # exp then reads a fully-written PSUM rectangle; the
                    # extra sub-diagonal columns of later tiles are never
                    # read by the (per-tile trimmed) PV matmuls.
                    i0g = max(0, 128 * grp[0] - qc0)
                    s_tile = spool.tile([128, kge * Nc], F32, tag="s", name="s_tile")
                    for tl, t in enumerate(grp):
                        nc.tensor.matmul(
                            s_tile[:, Nc * tl + i0g : Nc * tl + Nc],
                            lhsT=kT(t),
                            rhs=qT[:, qc0 - a + i0g : qc0 - a + Nc],
                            start=True,
                            stop=True,
                        )
                    return s_tile

                s_cur = emit_s(groups[0])
                flush(1)

                for gi, grp in enumerate(groups):
                    s_next = emit_s(groups[gi + 1]) if gi + 1 < len(groups) else None
                    proj_step()
                    ng = len(grp)
                    i0g = max(0, 128 * grp[0] - qc0)
                    p_tile = ppool.tile([128, kge * Nc], BF16, tag="p", name="p_tile")
                    if i0g == 0 or ng == 1:
                        s_ap = s_cur[:, i0g : (ng - 1) * Nc + Nc]
                        p_ap = p_tile[:, i0g : (ng - 1) * Nc + Nc]
                    else:
                        s_ap = s_cur.rearrange("p (t i) -> p t i", i=Nc)[
                            :, 0:ng, i0g:Nc
                        ]
                        p_ap = p_tile.rearrange("p (t i) -> p t i", i=Nc)[
                            :, 0:ng, i0g:Nc
                        ]
                    nc.scalar.activation(
                        p_ap,
                        s_ap,
                        mybir.ActivationFunctionType.Exp,
                        bias=zbias[:, :],
                        scale=SCALE,
                    )
                    for tl, t in enumerate(grp):
                        if qc0 <= 128 * t:  # diagonal block: triangular mask
                            dcol = 128 * t - qc0
                            blk = p_tile[:, Nc * tl + dcol : Nc * tl + dcol + 128]
                            nc.vector.tensor_tensor(
                                blk, blk, mask_sb[:, :], op=mybir.AluOpType.mult
                            )

                    def make_pv(
                        grp=grp, p_tile=p_tile, o_tile=o_tile, qc0=qc0, nqb=nqb, Nc=Nc
                    ):
                        def pv():
                            for tl, t in enumerate(grp):
                                for qb in range(nqb):
                                    gqb = qc0 // 128 + qb
                                    if t > gqb:
                                        continue
                                    c0p = Nc * tl + 128 * qb
                                    nc.tensor.matmul(
                                        o_tile[:, 65 * qb : 65 * qb + 65],
                                        lhsT=p_tile[:, c0p : c0p + 128],
                                        rhs=vx3[:, t, :],
                                        start=False,
                                        stop=(t == gqb),
                                        skip_group_check=True,
                                    )

                        return pv

                    pending.append(make_pv())
                    # at the very end nothing else can fill the pipeline:
                    # emit immediately rather than deferring into the tail
                    flush(0 if last_chunk and gi == len(groups) - 1 else 2)
                    s_cur = s_next

                def make_finish(
                    o_tile=o_tile, qc0=qc0, Nc=Nc, nqb=nqb, last=last_chunk
                ):
                    def fin():
                        o_sb = fpool.tile([128, 260], F32, tag="osb", name="o_sb")
                        nc.vector.tensor_copy(
                            o_sb[:, 0 : 65 * nqb], o_tile[:, 0 : 65 * nqb]
                        )
                        dst = o_d.ap()[qc0 - a : qc0 - a + Nc, :].rearrange(
                            "(qb p) e -> p qb e", p=128
                        )
                        # last chunk: HWDGE path on the now-idle SP queue
                        # (skips the ~1us SWDGE generation in the tail)
                        eng = nc.sync if last else nc.gpsimd
                        eng.dma_start(
                            dst,
                            o_sb.rearrange("p (qb e) -> p qb e", e=65)[:, 0:nqb, :],
                        )

                    return fin

                pending.append(make_finish())
            flush(0)

        # PSUM budget (8 banks): proj 3 + scores 2x2 + o 1
        prpool = stk.enter_context(tc.tile_pool(name="ppsum", bufs=3, space="PSUM"))
        spool = stk.enter_context(tc.tile_pool(name="spsum", bufs=2, space="PSUM"))
        opool = stk.enter_context(tc.tile_pool(name="opsum", bufs=1, space="PSUM"))
        attention(spool, opool, prpool)

    nc.compile()
    return nc


_cache = {}


def _programs():
    if "progs" not in _cache:
        _cache["progs"] = [
            build_shard(SHARDS[i], SHARDS[i + 1]) for i in range(len(SHARDS) - 1)
        ]
    return _cache["progs"]


def kernel(x, W_query, W_keys, W_value, _trace=False, _tracedir=None):
    progs = _programs()
    wqkv = np.concatenate([W_query, W_value, W_keys], axis=1).astype(np.float32)
    # packed constants: wqkv (c-major) | mask | ident (rows 0:64)
    consts = np.zeros((128, NDC * 192 + 192), np.float32)
    consts[:, 0 : NDC * 192] = (
        wqkv.reshape(NDC, 128, 192).transpose(1, 0, 2).reshape(128, NDC * 192)
    )
    consts[:, NDC * 192 : NDC * 192 + 128] = np.triu(np.ones((128, 128)))
    consts[0:64, NDC * 192 + 128 :] = np.eye(64)
    consts = consts.astype(ml_dtypes.bfloat16)
    xT = np.ascontiguousarray(np.transpose(x, (0, 2, 1))).astype(ml_dtypes.bfloat16)

    out = np.empty((B, N, D_OUT), np.float32)
    kT_acc = [np.zeros((64, 0), ml_dtypes.bfloat16) for _ in range(B)]
    vx_acc = [np.zeros((128, 0), ml_dtypes.bfloat16) for _ in range(B)]
    exec_ns = []
    kw = {}
    if _trace:
        kw = dict(trace=True, trace_cores=[0], tmpdir=_tracedir)
    for i, nc in enumerate(progs):
        a, bb = SHARDS[i], SHARDS[i + 1]
        in_maps = []
        for bi in range(B):
            m = {
                "xT": np.ascontiguousarray(xT[bi, :, a:bb]),
                "consts": consts,
            }
            if a:
                m["kT_in"] = np.ascontiguousarray(kT_acc[bi])
                m["vx_in"] = np.ascontiguousarray(vx_acc[bi])
            in_maps.append(m)
        core_ids = [0, 1, 2, 3] if i % 2 == 0 else [4, 5, 6, 7]
        res = run_bass_kernel_spmd(nc, in_maps, core_ids=core_ids, **kw)
        exec_ns.append(res.exec_time_ns)
        for bi in range(B):
            o = np.asarray(res.results[bi]["o"], dtype=np.float32)
            out[bi, a:bb] = o[:, :64] / o[:, 64:65]
            kT_acc[bi] = np.concatenate(
                [kT_acc[bi], np.asarray(res.results[bi]["kT_out"])], axis=1
            )
            vx_acc[bi] = np.concatenate(
                [vx_acc[bi], np.asarray(res.results[bi]["vx_out"])], axis=1
            )
    _cache["last_exec_ns"] = tuple(exec_ns)
    return out


# revision 79
# speedup vs baseline: 1.0265x; 1.0265x over previous
"""Causal attention kernel for Trainium2, 8 NeuronCores.

Problem: x[4,4096,768] f32; Wq/Wk/Wv [768,64] f32.
  q,k,v = x@W*; S = q@k.T (causal); out = softmax(S/8)@v -> [4,4096,64] f32.

Strategy: chained query-range shards, data-parallel over batch. The 4096
query rows split into contiguous ranges (SHARDS); launch i runs range i
for all 4 batches (one core per batch, alternating core groups 0-3/4-7).
Launches run back-to-back; each is an independently profiled program.

Per-shard device algorithm (q rows [a,b), keys [0,b)):
  - reads kT [64, a] (e-major) and vx [128, a/128*65] (token-major v with
    a ones column) for keys below its range from HBM -- written by the
    earlier shards -- and projects q/k/v only for its own [a,b) tokens.
  - scores transposed per (key-tile 128 x q-chunk) block on PE:
    ST[j,i] = sum_e kT[e,j] qT[e,i], causally trimmed.
  - P = exp(ST/8) via ScalarE into bf16 (no max subtraction: |S/8| small);
    diagonal 128x128 blocks masked by a triangular 0/1 mask on DVE.
  - attention output accumulated TRANSPOSED, one PSUM tile per q-block:
    o[q, 0:65] += P[k, q-block].T @ vx[k, 0:65]; the stationary operand is
    the P block, the 65-wide moving operand makes the PV matmuls cheap,
    and the ones column of vx accumulates the softmax denominators.
  - o[r, 65] f32 is DMA'd out token-major; the host does out = o[:,:64]/o[:,64:]
    (normalization only; no transposes).
"""

import numpy as np
import ml_dtypes

import concourse.bass as bass  # noqa: F401  (bacc pulls it in)
import concourse.bacc as bacc
import concourse.mybir as mybir
import concourse.tile as tile
from concourse.bass_utils import run_bass_kernel_spmd

B, N, D_IN, D_OUT = 4, 4096, 768, 64
NDC = D_IN // 128  # contraction chunks
BF16 = mybir.dt.bfloat16
F32 = mybir.dt.float32
SCALE = 1.0 / 8.0  # 1/sqrt(64)

# q-range boundaries of the shard chain (each a multiple of 128).
SHARDS = [0, 1024, 1664, 2176, 2560, 2944, 3328, 3584, 3840, 4096]


RAMP = (128, 256)


def _chunks_for(a, b, ramp):
    """q-chunk widths; small leading chunks let ScalarE start while the
    xT stream is still arriving."""
    out = []
    c0 = a
    if ramp:
        for w in RAMP:
            if c0 + w <= b:
                out.append((c0, w))
                c0 += w
    while c0 < b:
        w = min(512, b - c0)
        if w == 384:
            # widths must be powers of two: the score-strip sections are
            # packed at Nc stride, and a matmul output must not cross a
            # PSUM bank boundary (2KB); 384-wide sections would.
            w = 256
        out.append((c0, w))
        c0 += w
    return out


def _piece_bounds(a, b, ramp):
    """Token-piece boundaries for the xT load + projection groups."""
    bounds = [a]
    if ramp:
        for w in RAMP:
            if bounds[-1] + w <= b:
                bounds.append(bounds[-1] + w)
    while bounds[-1] < b:
        bounds.append(min(bounds[-1] + 512, b))
    return bounds


def build_shard(a, b):
    """Build the Bass program for q rows [a, b) (keys [0, b))."""
    r = b - a
    H = a // 128  # handoff key tiles
    TT = b // 128  # total key tiles
    nto = r // 128  # own key tiles
    KGRP = 2  # key tiles per score group (x2-buffered = 4 PSUM banks)
    ramp = True

    nc = bacc.Bacc("TRN2", target_bir_lowering=False, debug=False)

    xT_d = nc.dram_tensor("xT", [D_IN, r], BF16, kind="ExternalInput")
    const_d = nc.dram_tensor(
        "consts", [128, NDC * 192 + 192], BF16, kind="ExternalInput"
    )
    if a:
        kT_in_d = nc.dram_tensor("kT_in", [64, a], BF16, kind="ExternalInput")
        vx_in_d = nc.dram_tensor("vx_in", [128, H * 65], BF16, kind="ExternalInput")
    kT_out_d = nc.dram_tensor("kT_out", [64, r], BF16, kind="ExternalOutput")
    vx_out_d = nc.dram_tensor("vx_out", [128, nto * 65], BF16, kind="ExternalOutput")
    o_d = nc.dram_tensor("o", [r, 65], F32, kind="ExternalOutput")

    from contextlib import ExitStack

    with tile.TileContext(nc) as tc, ExitStack() as stk:
        cpool = stk.enter_context(tc.tile_pool(name="const", bufs=1))
        xpool = stk.enter_context(tc.tile_pool(name="xt", bufs=1))
        jpool = stk.enter_context(tc.tile_pool(name="proj", bufs=1))
        ppool = stk.enter_context(tc.tile_pool(name="pp", bufs=4))
        fpool = stk.enter_context(tc.tile_pool(name="fin", bufs=2))

        # ---- constants (one packed load: wqkv | mask | ident) ----
        const_sb = cpool.tile([128, NDC * 192 + 192], BF16, tag="const")
        w3 = const_sb[:, 0 : NDC * 192].rearrange("p (c j) -> p c j", j=192)
        mask_sb = const_sb[:, NDC * 192 : NDC * 192 + 128]
        ident_sb = const_sb[0:64, NDC * 192 + 128 : NDC * 192 + 192]
        zbias = cpool.tile([128, 1], F32, tag="zbias")
        nc.vector.memset(zbias[:, :], 0.0)
        zeros_sb = cpool.tile([128, 260], BF16, tag="zeros")
        nc.vector.memset(zeros_sb[:, :], 0.0)

        # ---- SBUF buffers ----
        xt_sb = xpool.tile([128, NDC * r], BF16, tag="xt")
        xt3 = xt_sb.rearrange("p (c n) -> p c n", n=r)
        xT3d = xT_d.ap().rearrange("(c p) n -> p c n", p=128)
        # kq band: rows 0:64 vT (own cols), rows 64:128 kT (cols 0:b) and
        # qT (cols b:b+r). kT and qT share base partition 64 for the
        # S-matmul.
        kq_sb = jpool.tile([128, b + r], BF16, tag="kq")
        vx_sb = jpool.tile([128, TT * 65], BF16, tag="vx")
        vx3 = vx_sb.rearrange("p (t e) -> p t e", e=65)
        # softmax-denominator ones column for all own tiles, set once
        # (handoff tiles arrive from HBM with their ones already set)
        nc.vector.memset(vx3[:, H:TT, 64:65], 1.0)

        qT = kq_sb[64:128, b : b + r]

        def kT(t):
            return kq_sb[64:128, 128 * t : 128 * (t + 1)]

        # ---- input DMAs (interleaved so early pieces land first) ----
        xb = _piece_bounds(a, b, ramp)
        xt_pieces = list(zip(xb[:-1], xb[1:]))
        kv_pieces = []
        if a:
            kb = [0]
            while kb[-1] < a:
                kb.append(min(kb[-1] + (512 if len(kb) < 3 else 1024), a))
            kv_pieces = list(zip(kb[:-1], kb[1:]))

        # input loads: the HWDGE device is shared across SP/ACT queues and
        # holds ~630ns per DMA, so keep the critical-path loads (consts,
        # xT stream, first kT piece) on HWDGE queues; everything else goes
        # via the gpsimd SWDGE path, which generates descriptors on the
        # otherwise-idle Pool engine. The kT stream (128B/key) is kept
        # separate from the 3x-bigger vx payload so scores never starve.
        nc.sync.dma_start(const_sb[:, :], const_d.ap())
        for g0, g1 in xt_pieces:
            nc.sync.dma_start(xt3[:, :, g0 - a : g1 - a], xT3d[:, :, g0 - a : g1 - a])
        for i, (k0, k1) in enumerate(kv_pieces):
            eng = nc.scalar if i == 0 else nc.gpsimd
            eng.dma_start(kq_sb[64:128, k0:k1], kT_in_d.ap()[:, k0:k1])
        if a:
            vx3d = vx_in_d.ap().rearrange("p (t e) -> p t e", e=65)
            h2 = max(1, H // 2)
            nc.gpsimd.dma_start(vx3[:, 0:h2, :], vx3d[:, 0:h2, :])
            if h2 < H:
                nc.gpsimd.dma_start(vx3[:, h2:H, :], vx3d[:, h2:H, :])

        # ---- projections ----
        done = {"q": a, "kv": a}
        bset = sorted(set(xb))

        def _grp_end(g0):
            import bisect

            i = bisect.bisect_right(bset, g0)
            return bset[i] if i < len(bset) else b

        def emit_q_upto(tok, pool):
            while done["q"] < min(tok, b):
                g0 = done["q"]
                g = min(_grp_end(g0), b) - g0
                ps = pool.tile([128, 512], F32, tag="proj", name="ps")
                for dc in range(NDC):
                    nc.tensor.matmul(
                        ps[64:128, 0:g],
                        lhsT=w3[:, dc, 0:64],
                        rhs=xt3[:, dc, g0 - a : g0 - a + g],
                        start=(dc == 0),
                        stop=(dc == NDC - 1),
                        tile_position=(0, 64),
                    )
                nc.vector.tensor_copy(
                    kq_sb[64:128, b + g0 - a : b + g0 - a + g], ps[64:128, 0:g]
                )
                done["q"] = g0 + g

        def emit_vx(t0, t1, pool):
            # v -> token-major on the PE (cheap 64-row transpose matmuls;
            # keeps the chain off the DMA/HWDGE queues), then one DVE copy
            # into the vx slots. The ones column was memset up front.
            tp = pool.tile([128, 512], BF16, tag="proj", name="tp")
            for ti in range(t1 - t0):
                nc.tensor.transpose(
                    tp[:, 64 * ti : 64 * ti + 64],
                    kq_sb[0:64, 128 * (t0 + ti) : 128 * (t0 + ti + 1)],
                    ident_sb[:, :],
                )
            nc.vector.tensor_copy(
                vx3[:, t0:t1, 0:64],
                tp.rearrange("p (t e) -> p t e", e=64)[:, 0 : t1 - t0, :],
            )
            # handoff to later shards
            nc.gpsimd.dma_start(
                kT_out_d.ap()[:, 128 * t0 - a : 128 * t1 - a],
                kq_sb[64:128, 128 * t0 : 128 * t1],
            )
            nc.gpsimd.dma_start(
                vx_out_d.ap().rearrange("p (t e) -> p t e", e=65)[
                    :, t0 - H : t1 - H, :
                ],
                vx3[:, t0:t1, :],
            )

        def emit_kv_upto(tok, pool):
            while done["kv"] < min(tok, b):
                g0 = done["kv"]
                g = min(_grp_end(g0), b) - g0
                ps = pool.tile([128, 512], F32, tag="proj", name="ps")
                for dc in range(NDC):
                    nc.tensor.matmul(
                        ps[:, 0:g],
                        lhsT=w3[:, dc, 64:192],
                        rhs=xt3[:, dc, g0 - a : g0 - a + g],
                        start=(dc == 0),
                        stop=(dc == NDC - 1),
                    )
                nc.vector.tensor_copy(kq_sb[:, g0 : g0 + g], ps[:, 0:g])
                done["kv"] = g0 + g
                # defer the v-transpose one step so its dep (the kv copy)
                # is met by the time it reaches the in-order PE sequencer
                t0, t1 = g0 // 128, (g0 + g) // 128
                if t1 > t0:
                    if done.get("vxp") is not None:
                        done["vxp"]()
                    done["vxp"] = lambda t0=t0, t1=t1: emit_vx(t0, t1, pool)

        def flush_vx():
            if done.get("vxp") is not None:
                done["vxp"]()
                done["vxp"] = None

        # partial-tile leftovers are impossible: bounds are 128-aligned

        # ---- attention ----
        def attention(spool, opool, pool_for_proj):
            chunks = _chunks_for(a, b, ramp)
            # deferred closures (PV of an earlier group / o drains): emitted
            # one group late so their deps are met when they reach the PE
            # sequencer -- parked instructions fill the 4-deep wait queue and
            # stall everything behind them.
            pending = []

            def flush(keep=0):
                while len(pending) > keep:
                    pending.pop(0)()

            for ci, (qc0, Nc) in enumerate(chunks):
                last_chunk = ci == len(chunks) - 1
                nqb = Nc // 128
                T_c = (qc0 + Nc) // 128
                # narrow chunks pack more key tiles per PSUM slot / exp call
                kge = max(KGRP, (KGRP * 512) // Nc)
                if pool_for_proj is not None:
                    # hard guards (normally no-ops: the per-group proj steps
                    # below keep projections ahead of their consumers)
                    emit_q_upto(qc0 + Nc, pool_for_proj)
                    emit_kv_upto(T_c * 128, pool_for_proj)
                    flush_vx()

                def proj_step(qc0=qc0, Nc=Nc):
                    # advance up to one q and one kv projection group per
                    # attention group: spreads proj matmuls through the PE
                    # stream so they fill ACT-bound bubbles without parking
                    # the in-order sequencer on the proj PSUM pool. kv leads
                    # by a chunk so the vx chain stays ahead of its PV use.
                    if pool_for_proj is None:
                        return
                    if done["q"] < min(qc0 + Nc + 512, b):
                        emit_q_upto(done["q"] + 1, pool_for_proj)
                    if done["kv"] < min(qc0 + Nc + 1024, b):
                        emit_kv_upto(done["kv"] + 1, pool_for_proj)
                tiles = list(range(T_c))
                groups = [tiles[t0 : t0 + kge] for t0 in range(0, T_c, kge)]
                o_tile = opool.tile([128, 260], F32, tag="o", name="o_tile")
                # single full-width start for the whole o tile: a matmul's
                # start=True clears has_written for the entire PSUM bank, so
                # per-q-block chains must NOT each open their own group --
                # later starts would flip earlier chains' columns back to
                # overwrite mode and drop their first-tile contributions.
                nc.tensor.matmul(
                    o_tile[:, 0 : 65 * nqb],
                    lhsT=zeros_sb[:, 0:128],
                    rhs=zeros_sb[:, 0 : 65 * nqb],
                    start=True,
                    stop=False,
                    skip_group_check=True,
                )

                def emit_s(grp, qc0=qc0, Nc=Nc):
                    # all tiles of the group write cols [i0g, Nc): the ACT
                    # exp then reads a fully-written PSUM rectangle; the
                    # extra sub-diagonal columns of later tiles are never
                    # read by the (per-tile trimmed) PV matmuls.
                    i0g = max(0, 128 * grp[0] - qc0)
                    s_tile = spool.tile([128, kge * Nc], F32, tag="s", name="s_tile")
                    for tl, t in enumerate(grp):
                        nc.tensor.matmul(
                            s_tile[:, Nc * tl + i0g : Nc * tl + Nc],
                            lhsT=kT(t),
                            rhs=qT[:, qc0 - a + i0g : qc0 - a + Nc],
                            start=True,
                            stop=True,
                        )
                    return s_tile

                s_cur = emit_s(groups[0])
                flush(1)

                for gi, grp in enumerate(groups):
                    s_next = emit_s(groups[gi + 1]) if gi + 1 < len(groups) else None
                    proj_step()
                    ng = len(grp)
                    i0g = max(0, 128 * grp[0] - qc0)
                    p_tile = ppool.tile([128, kge * Nc], BF16, tag="p", name="p_tile")
                    if i0g == 0 or ng == 1:
                        s_ap = s_cur[:, i0g : (ng - 1) * Nc + Nc]
                        p_ap = p_tile[:, i0g : (ng - 1) * Nc + Nc]
                    else:
                        s_ap = s_cur.rearrange("p (t i) -> p t i", i=Nc)[
                            :, 0:ng, i0g:Nc
                        ]
                        p_ap = p_tile.rearrange("p (t i) -> p t i", i=Nc)[
                            :, 0:ng, i0g:Nc
                        ]
                    nc.scalar.activation(
                        p_ap,
                        s_ap,
                        mybir.ActivationFunctionType.Exp,
                        bias=zbias[:, :],
                        scale=SCALE,
                    )
                    for tl, t in enumerate(grp):
                        if qc0 <= 128 * t:  # diagonal block: triangular mask
                            dcol = 128 * t - qc0
                            blk = p_tile[:, Nc * tl + dcol : Nc * tl + dcol + 128]
                            nc.vector.tensor_tensor(
                                blk, blk, mask_sb[:, :], op=mybir.AluOpType.mult
                            )

                    def make_pv(
                        grp=grp, p_tile=p_tile, o_tile=o_tile, qc0=qc0, nqb=nqb, Nc=Nc
                    ):
                        def pv():
                            for tl, t in enumerate(grp):
                                for qb in range(nqb):
                                    gqb = qc0 // 128 + qb
                                    if t > gqb:
                                        continue
                                    c0p = Nc * tl + 128 * qb
                                    nc.tensor.matmul(
                                        o_tile[:, 65 * qb : 65 * qb + 65],
                                        lhsT=p_tile[:, c0p : c0p + 128],
                                        rhs=vx3[:, t, :],
                                        start=False,
                                        stop=(t == gqb),
                                        skip_group_check=True,
                                    )

                        return pv

                    pending.append(make_pv())
                    # at the very end nothing else can fill the pipeline:
                    # emit immediately rather than deferring into the tail
                    flush(0 if last_chunk and gi == len(groups) - 1 else 2)
                    s_cur = s_next

                def make_finish(
                    o_tile=o_tile, qc0=qc0, Nc=Nc, nqb=nqb, last=last_chunk
                ):
                    def fin():
                        o_sb = fpool.tile([128, 260], F32, tag="osb", name="o_sb")
                        nc.vector.tensor_copy(
                            o_sb[:, 0 : 65 * nqb], o_tile[:, 0 : 65 * nqb]
                        )
                        dst = o_d.ap()[qc0 - a : qc0 - a + Nc, :].rearrange(
                            "(qb p) e -> p qb e", p=128
                        )
                        # last chunk: HWDGE path on the now-idle SP queue
                        # (skips the ~1us SWDGE generation in the tail)
                        eng = nc.sync if last else nc.gpsimd
                        eng.dma_start(
                            dst,
                            o_sb.rearrange("p (qb e) -> p qb e", e=65)[:, 0:nqb, :],
                        )

                    return fin

                pending.append(make_finish())
            flush(0)

        # PSUM budget (8 banks): proj 3 + scores 2x2 + o 1
        prpool = stk.enter_context(tc.tile_pool(name="ppsum", bufs=3, space="PSUM"))
        spool = stk.enter_context(tc.tile_pool(name="spsum", bufs=2, space="PSUM"))
        opool = stk.enter_context(tc.tile_pool(name="opsum", bufs=1, space="PSUM"))
        attention(spool, opool, prpool)

    nc.compile()
    return nc


_cache = {}


def _programs():
    if "progs" not in _cache:
        _cache["progs"] = [
            build_shard(SHARDS[i], SHARDS[i + 1]) for i in range(len(SHARDS) - 1)
        ]
    return _cache["progs"]


def kernel(x, W_query, W_keys, W_value, _trace=False, _tracedir=None):
    progs = _programs()
    wqkv = np.concatenate([W_query, W_value, W_keys], axis=1).astype(np.float32)
    # packed constants: wqkv (c-major) | mask | ident (rows 0:64)
    consts = np.zeros((128, NDC * 192 + 192), np.float32)
    consts[:, 0 : NDC * 192] = (
        wqkv.reshape(NDC, 128, 192).transpose(1, 0, 2).reshape(128, NDC * 192)
    )
    consts[:, NDC * 192 : NDC * 192 + 128] = np.triu(np.ones((128, 128)))
    consts[0:64, NDC * 192 + 128 :] = np.eye(64)
    consts = consts.astype(ml_dtypes.bfloat16)
    xT = np.ascontiguousarray(np.transpose(x, (0, 2, 1))).astype(ml_dtypes.bfloat16)

    out = np.empty((B, N, D_OUT), np.float32)
    kT_acc = [np.zeros((64, 0), ml_dtypes.bfloat16) for _ in range(B)]
    vx_acc = [np.zeros((128, 0), ml_dtypes.bfloat16) for _ in range(B)]
    exec_ns = []
    kw = {}
    if _trace:
        kw = dict(trace=True, trace_cores=[0], tmpdir=_tracedir)
    for i, nc in enumerate(progs):
        a, bb = SHARDS[i], SHARDS[i + 1]
        in_maps = []
        for bi in range(B):
            m = {
                "xT": np.ascontiguousarray(xT[bi, :, a:bb]),
                "consts": consts,
            }
            if a:
                m["kT_in"] = np.ascontiguousarray(kT_acc[bi])
                m["vx_in"] = np.ascontiguousarray(vx_acc[bi])
            in_maps.append(m)
        core_ids = [0, 1, 2, 3] if i % 2 == 0 else [4, 5, 6, 7]
        res = run_bass_kernel_spmd(nc, in_maps, core_ids=core_ids, **kw)
        exec_ns.append(res.exec_time_ns)
        for bi in range(B):
            o = np.asarray(res.results[bi]["o"], dtype=np.float32)
            out[bi, a:bb] = o[:, :64] / o[:, 64:65]
            kT_acc[bi] = np.concatenate(
                [kT_acc[bi], np.asarray(res.results[bi]["kT_out"])], axis=1
            )
            vx_acc[bi] = np.concatenate(
                [vx_acc[bi], np.asarray(res.results[bi]["vx_out"])], axis=1
            )
    _cache["last_exec_ns"] = tuple(exec_ns)
    return out


# revision 80
# speedup vs baseline: 1.1127x; 1.0840x over previous
"""Causal attention kernel for Trainium2, 8 NeuronCores.

Problem: x[4,4096,768] f32; Wq/Wk/Wv [768,64] f32.
  q,k,v = x@W*; S = q@k.T (causal); out = softmax(S/8)@v -> [4,4096,64] f32.

Strategy: chained query-range shards, data-parallel over batch. The 4096
query rows split into contiguous ranges (SHARDS); launch i runs range i
for all 4 batches (one core per batch, alternating core groups 0-3/4-7).
Launches run back-to-back; each is an independently profiled program.

Per-shard device algorithm (q rows [a,b), keys [0,b)):
  - reads kT [64, a] (e-major) and vx [128, a/128*65] (token-major v with
    a ones column) for keys below its range from HBM -- written by the
    earlier shards -- and projects q/k/v only for its own [a,b) tokens.
  - scores transposed per (key-tile 128 x q-chunk) block on PE:
    ST[j,i] = sum_e kT[e,j] qT[e,i], causally trimmed.
  - P = exp(ST/8) via ScalarE into bf16 (no max subtraction: |S/8| small);
    diagonal 128x128 blocks masked by a triangular 0/1 mask on DVE.
  - attention output accumulated TRANSPOSED, one PSUM tile per q-block:
    o[q, 0:65] += P[k, q-block].T @ vx[k, 0:65]; the stationary operand is
    the P block, the 65-wide moving operand makes the PV matmuls cheap,
    and the ones column of vx accumulates the softmax denominators.
  - o[r, 65] f32 is DMA'd out token-major; the host does out = o[:,:64]/o[:,64:]
    (normalization only; no transposes).
"""

import numpy as np
import ml_dtypes

import concourse.bass as bass  # noqa: F401  (bacc pulls it in)
import concourse.bacc as bacc
import concourse.mybir as mybir
import concourse.tile as tile
from concourse.bass_utils import run_bass_kernel_spmd

B, N, D_IN, D_OUT = 4, 4096, 768, 64
NDC = D_IN // 128  # contraction chunks
BF16 = mybir.dt.bfloat16
F32 = mybir.dt.float32
SCALE = 1.0 / 8.0  # 1/sqrt(64)

# q-range boundaries of the shard chain (each a multiple of 128).
SHARDS = [0, 896, 1536, 2048, 2432, 2816, 3072, 3328, 3584, 3840, 4096]


RAMP = (128, 256)


def _chunks_for(a, b, ramp):
    """q-chunk widths; small leading chunks let ScalarE start while the
    xT stream is still arriving."""
    out = []
    c0 = a
    if ramp:
        for w in RAMP:
            if c0 + w <= b:
                out.append((c0, w))
                c0 += w
    while c0 < b:
        w = min(512, b - c0)
        if w == 384:
            # widths must be powers of two: the score-strip sections are
            # packed at Nc stride, and a matmul output must not cross a
            # PSUM bank boundary (2KB); 384-wide sections would.
            w = 256
        out.append((c0, w))
        c0 += w
    return out


def _piece_bounds(a, b, ramp):
    """Token-piece boundaries for the xT load + projection groups."""
    bounds = [a]
    if ramp:
        for w in RAMP:
            if bounds[-1] + w <= b:
                bounds.append(bounds[-1] + w)
    while bounds[-1] < b:
        bounds.append(min(bounds[-1] + 512, b))
    return bounds


def build_shard(a, b):
    """Build the Bass program for q rows [a, b) (keys [0, b))."""
    r = b - a
    H = a // 128  # handoff key tiles
    TT = b // 128  # total key tiles
    nto = r // 128  # own key tiles
    KGRP = 2  # key tiles per score group (x2-buffered = 4 PSUM banks)
    ramp = True

    nc = bacc.Bacc("TRN2", target_bir_lowering=False, debug=False)

    xT_d = nc.dram_tensor("xT", [D_IN, r], BF16, kind="ExternalInput")
    const_d = nc.dram_tensor(
        "consts", [128, NDC * 192 + 192], BF16, kind="ExternalInput"
    )
    if a:
        kT_in_d = nc.dram_tensor("kT_in", [64, a], BF16, kind="ExternalInput")
        vx_in_d = nc.dram_tensor("vx_in", [128, H * 65], BF16, kind="ExternalInput")
    kT_out_d = nc.dram_tensor("kT_out", [64, r], BF16, kind="ExternalOutput")
    vx_out_d = nc.dram_tensor("vx_out", [128, nto * 65], BF16, kind="ExternalOutput")
    o_d = nc.dram_tensor("o", [r, 65], F32, kind="ExternalOutput")

    from contextlib import ExitStack

    with tile.TileContext(nc) as tc, ExitStack() as stk:
        cpool = stk.enter_context(tc.tile_pool(name="const", bufs=1))
        xpool = stk.enter_context(tc.tile_pool(name="xt", bufs=1))
        jpool = stk.enter_context(tc.tile_pool(name="proj", bufs=1))
        ppool = stk.enter_context(tc.tile_pool(name="pp", bufs=4))
        fpool = stk.enter_context(tc.tile_pool(name="fin", bufs=2))

        # ---- constants (one packed load: wqkv | mask | ident) ----
        const_sb = cpool.tile([128, NDC * 192 + 192], BF16, tag="const")
        w3 = const_sb[:, 0 : NDC * 192].rearrange("p (c j) -> p c j", j=192)
        mask_sb = const_sb[:, NDC * 192 : NDC * 192 + 128]
        ident_sb = const_sb[0:64, NDC * 192 + 128 : NDC * 192 + 192]
        zbias = cpool.tile([128, 1], F32, tag="zbias")
        nc.vector.memset(zbias[:, :], 0.0)
        zeros_sb = cpool.tile([128, 260], BF16, tag="zeros")
        nc.vector.memset(zeros_sb[:, :], 0.0)

        # ---- SBUF buffers ----
        xt_sb = xpool.tile([128, NDC * r], BF16, tag="xt")
        xt3 = xt_sb.rearrange("p (c n) -> p c n", n=r)
        xT3d = xT_d.ap().rearrange("(c p) n -> p c n", p=128)
        # kq band: rows 0:64 vT (own cols), rows 64:128 kT (cols 0:b) and
        # qT (cols b:b+r). kT and qT share base partition 64 for the
        # S-matmul.
        kq_sb = jpool.tile([128, b + r], BF16, tag="kq")
        vx_sb = jpool.tile([128, TT * 65], BF16, tag="vx")
        vx3 = vx_sb.rearrange("p (t e) -> p t e", e=65)
        # softmax-denominator ones column for all own tiles, set once
        # (handoff tiles arrive from HBM with their ones already set)
        nc.vector.memset(vx3[:, H:TT, 64:65], 1.0)

        qT = kq_sb[64:128, b : b + r]

        def kT(t):
            return kq_sb[64:128, 128 * t : 128 * (t + 1)]

        # ---- input DMAs (interleaved so early pieces land first) ----
        xb = _piece_bounds(a, b, ramp)
        xt_pieces = list(zip(xb[:-1], xb[1:]))
        kv_pieces = []
        if a:
            kb = [0]
            while kb[-1] < a:
                kb.append(min(kb[-1] + (512 if len(kb) < 3 else 1024), a))
            kv_pieces = list(zip(kb[:-1], kb[1:]))

        # input loads: the HWDGE device is shared across SP/ACT queues and
        # holds ~630ns per DMA, so keep the critical-path loads (consts,
        # xT stream, first kT piece) on HWDGE queues; everything else goes
        # via the gpsimd SWDGE path, which generates descriptors on the
        # otherwise-idle Pool engine. The kT stream (128B/key) is kept
        # separate from the 3x-bigger vx payload so scores never starve.
        nc.sync.dma_start(const_sb[:, :], const_d.ap())
        for g0, g1 in xt_pieces:
            nc.sync.dma_start(xt3[:, :, g0 - a : g1 - a], xT3d[:, :, g0 - a : g1 - a])
        for i, (k0, k1) in enumerate(kv_pieces):
            eng = nc.scalar if i == 0 else nc.gpsimd
            eng.dma_start(kq_sb[64:128, k0:k1], kT_in_d.ap()[:, k0:k1])
        if a:
            vx3d = vx_in_d.ap().rearrange("p (t e) -> p t e", e=65)
            h2 = max(1, H // 2)
            nc.gpsimd.dma_start(vx3[:, 0:h2, :], vx3d[:, 0:h2, :])
            if h2 < H:
                nc.gpsimd.dma_start(vx3[:, h2:H, :], vx3d[:, h2:H, :])

        # ---- projections ----
        done = {"q": a, "kv": a}
        bset = sorted(set(xb))

        def _grp_end(g0):
            import bisect

            i = bisect.bisect_right(bset, g0)
            return bset[i] if i < len(bset) else b

        def emit_q_upto(tok, pool):
            while done["q"] < min(tok, b):
                g0 = done["q"]
                g = min(_grp_end(g0), b) - g0
                ps = pool.tile([128, 512], F32, tag="proj", name="ps")
                for dc in range(NDC):
                    nc.tensor.matmul(
                        ps[64:128, 0:g],
                        lhsT=w3[:, dc, 0:64],
                        rhs=xt3[:, dc, g0 - a : g0 - a + g],
                        start=(dc == 0),
                        stop=(dc == NDC - 1),
                        tile_position=(0, 64),
                    )
                nc.vector.tensor_copy(
                    kq_sb[64:128, b + g0 - a : b + g0 - a + g], ps[64:128, 0:g]
                )
                done["q"] = g0 + g

        def emit_vx(t0, t1, pool):
            # v -> token-major on the PE (cheap 64-row transpose matmuls;
            # keeps the chain off the DMA/HWDGE queues), then one DVE copy
            # into the vx slots. The ones column was memset up front.
            tp = pool.tile([128, 512], BF16, tag="proj", name="tp")
            for ti in range(t1 - t0):
                nc.tensor.transpose(
                    tp[:, 64 * ti : 64 * ti + 64],
                    kq_sb[0:64, 128 * (t0 + ti) : 128 * (t0 + ti + 1)],
                    ident_sb[:, :],
                )
            nc.vector.tensor_copy(
                vx3[:, t0:t1, 0:64],
                tp.rearrange("p (t e) -> p t e", e=64)[:, 0 : t1 - t0, :],
            )
            # handoff to later shards
            nc.gpsimd.dma_start(
                kT_out_d.ap()[:, 128 * t0 - a : 128 * t1 - a],
                kq_sb[64:128, 128 * t0 : 128 * t1],
            )
            nc.gpsimd.dma_start(
                vx_out_d.ap().rearrange("p (t e) -> p t e", e=65)[
                    :, t0 - H : t1 - H, :
                ],
                vx3[:, t0:t1, :],
            )

        def emit_kv_upto(tok, pool):
            while done["kv"] < min(tok, b):
                g0 = done["kv"]
                g = min(_grp_end(g0), b) - g0
                ps = pool.tile([128, 512], F32, tag="proj", name="ps")
                for dc in range(NDC):
                    nc.tensor.matmul(
                        ps[:, 0:g],
                        lhsT=w3[:, dc, 64:192],
                        rhs=xt3[:, dc, g0 - a : g0 - a + g],
                        start=(dc == 0),
                        stop=(dc == NDC - 1),
                    )
                nc.vector.tensor_copy(kq_sb[:, g0 : g0 + g], ps[:, 0:g])
                done["kv"] = g0 + g
                # defer the v-transpose one step so its dep (the kv copy)
                # is met by the time it reaches the in-order PE sequencer
                t0, t1 = g0 // 128, (g0 + g) // 128
                if t1 > t0:
                    if done.get("vxp") is not None:
                        done["vxp"]()
                    done["vxp"] = lambda t0=t0, t1=t1: emit_vx(t0, t1, pool)

        def flush_vx():
            if done.get("vxp") is not None:
                done["vxp"]()
                done["vxp"] = None

        # partial-tile leftovers are impossible: bounds are 128-aligned

        # ---- attention ----
        def attention(spool, opool, pool_for_proj):
            chunks = _chunks_for(a, b, ramp)
            # deferred closures (PV of an earlier group / o drains): emitted
            # one group late so their deps are met when they reach the PE
            # sequencer -- parked instructions fill the 4-deep wait queue and
            # stall everything behind them.
            pending = []

            def flush(keep=0):
                while len(pending) > keep:
                    pending.pop(0)()

            for ci, (qc0, Nc) in enumerate(chunks):
                last_chunk = ci == len(chunks) - 1
                nqb = Nc // 128
                T_c = (qc0 + Nc) // 128
                # narrow chunks pack more key tiles per PSUM slot / exp call
                kge = max(KGRP, (KGRP * 512) // Nc)
                if pool_for_proj is not None:
                    # hard guards (normally no-ops: the per-group proj steps
                    # below keep projections ahead of their consumers)
                    emit_q_upto(qc0 + Nc, pool_for_proj)
                    emit_kv_upto(T_c * 128, pool_for_proj)
                    flush_vx()

                def proj_step(qc0=qc0, Nc=Nc):
                    # advance up to one q and one kv projection group per
                    # attention group: spreads proj matmuls through the PE
                    # stream so they fill ACT-bound bubbles without parking
                    # the in-order sequencer on the proj PSUM pool. kv leads
                    # by a chunk so the vx chain stays ahead of its PV use.
                    if pool_for_proj is None:
                        return
                    if done["q"] < min(qc0 + Nc + 512, b):
                        emit_q_upto(done["q"] + 1, pool_for_proj)
                    if done["kv"] < min(qc0 + Nc + 1024, b):
                        emit_kv_upto(done["kv"] + 1, pool_for_proj)
                tiles = list(range(T_c))
                groups = [tiles[t0 : t0 + kge] for t0 in range(0, T_c, kge)]
                o_tile = opool.tile([128, 260], F32, tag="o", name="o_tile")
                # single full-width start for the whole o tile: a matmul's
                # start=True clears has_written for the entire PSUM bank, so
                # per-q-block chains must NOT each open their own group --
                # later starts would flip earlier chains' columns back to
                # overwrite mode and drop their first-tile contributions.
                nc.tensor.matmul(
                    o_tile[:, 0 : 65 * nqb],
                    lhsT=zeros_sb[:, 0:128],
                    rhs=zeros_sb[:, 0 : 65 * nqb],
                    start=True,
                    stop=False,
                    skip_group_check=True,
                )

                def emit_s(grp, qc0=qc0, Nc=Nc):
                    # all tiles of the group write cols [i0g, Nc): the ACT
                    # exp then reads a fully-written PSUM rectangle; the
                    # extra sub-diagonal columns of later tiles are never
                    # read by the (per-tile trimmed) PV matmuls.
                    i0g = max(0, 128 * grp[0] - qc0)
                    s_tile = spool.tile([128, kge * Nc], F32, tag="s", name="s_tile")
                    for tl, t in enumerate(grp):
                        nc.tensor.matmul(
                            s_tile[:, Nc * tl + i0g : Nc * tl + Nc],
                            lhsT=kT(t),
                            rhs=qT[:, qc0 - a + i0g : qc0 - a + Nc],
                            start=True,
                            stop=True,
                        )
                    return s_tile

                s_cur = emit_s(groups[0])
                flush(1)

                for gi, grp in enumerate(groups):
                    s_next = emit_s(groups[gi + 1]) if gi + 1 < len(groups) else None
                    proj_step()
                    ng = len(grp)
                    i0g = max(0, 128 * grp[0] - qc0)
                    p_tile = ppool.tile([128, kge * Nc], BF16, tag="p", name="p_tile")
                    if i0g == 0 or ng == 1:
                        s_ap = s_cur[:, i0g : (ng - 1) * Nc + Nc]
                        p_ap = p_tile[:, i0g : (ng - 1) * Nc + Nc]
                    else:
                        s_ap = s_cur.rearrange("p (t i) -> p t i", i=Nc)[
                            :, 0:ng, i0g:Nc
                        ]
                        p_ap = p_tile.rearrange("p (t i) -> p t i", i=Nc)[
                            :, 0:ng, i0g:Nc
                        ]
                    nc.scalar.activation(
                        p_ap,
                        s_ap,
                        mybir.ActivationFunctionType.Exp,
                        bias=zbias[:, :],
                        scale=SCALE,
                    )
                    for tl, t in enumerate(grp):
                        if qc0 <= 128 * t:  # diagonal block: triangular mask
                            dcol = 128 * t - qc0
                            blk = p_tile[:, Nc * tl + dcol : Nc * tl + dcol + 128]
                            nc.vector.tensor_tensor(
                                blk, blk, mask_sb[:, :], op=mybir.AluOpType.mult
                            )

                    def make_pv(
                        grp=grp, p_tile=p_tile, o_tile=o_tile, qc0=qc0, nqb=nqb, Nc=Nc
                    ):
                        def pv():
                            for tl, t in enumerate(grp):
                                for qb in range(nqb):
                                    gqb = qc0 // 128 + qb
                                    if t > gqb:
                                        continue
                                    c0p = Nc * tl + 128 * qb
                                    nc.tensor.matmul(
                                        o_tile[:, 65 * qb : 65 * qb + 65],
                                        lhsT=p_tile[:, c0p : c0p + 128],
                                        rhs=vx3[:, t, :],
                                        start=False,
                                        stop=(t == gqb),
                                        skip_group_check=True,
                                    )

                        return pv

                    pending.append(make_pv())
                    # at the very end nothing else can fill the pipeline:
                    # emit immediately rather than deferring into the tail
                    flush(0 if last_chunk and gi == len(groups) - 1 else 2)
                    s_cur = s_next

                def make_finish(
                    o_tile=o_tile, qc0=qc0, Nc=Nc, nqb=nqb, last=last_chunk
                ):
                    def fin():
                        o_sb = fpool.tile([128, 260], F32, tag="osb", name="o_sb")
                        nc.vector.tensor_copy(
                            o_sb[:, 0 : 65 * nqb], o_tile[:, 0 : 65 * nqb]
                        )
                        dst = o_d.ap()[qc0 - a : qc0 - a + Nc, :].rearrange(
                            "(qb p) e -> p qb e", p=128
                        )
                        # last chunk: HWDGE path on the now-idle SP queue
                        # (skips the ~1us SWDGE generation in the tail)
                        eng = nc.sync if last else nc.gpsimd
                        eng.dma_start(
                            dst,
                            o_sb.rearrange("p (qb e) -> p qb e", e=65)[:, 0:nqb, :],
                        )

                    return fin

                pending.append(make_finish())
            flush(0)

        # PSUM budget (8 banks): proj 3 + scores 2x2 + o 1
        prpool = stk.enter_context(tc.tile_pool(name="ppsum", bufs=3, space="PSUM"))
        spool = stk.enter_context(tc.tile_pool(name="spsum", bufs=2, space="PSUM"))
        opool = stk.enter_context(tc.tile_pool(name="opsum", bufs=1, space="PSUM"))
        attention(spool, opool, prpool)

    nc.compile()
    return nc


_cache = {}


def _programs():
    if "progs" not in _cache:
        _cache["progs"] = [
            build_shard(SHARDS[i], SHARDS[i + 1]) for i in range(len(SHARDS) - 1)
        ]
    return _cache["progs"]


def kernel(x, W_query, W_keys, W_value, _trace=False, _tracedir=None):
    progs = _programs()
    wqkv = np.concatenate([W_query, W_value, W_keys], axis=1).astype(np.float32)
    # packed constants: wqkv (c-major) | mask | ident (rows 0:64)
    consts = np.zeros((128, NDC * 192 + 192), np.float32)
    consts[:, 0 : NDC * 192] = (
        wqkv.reshape(NDC, 128, 192).transpose(1, 0, 2).reshape(128, NDC * 192)
    )
    consts[:, NDC * 192 : NDC * 192 + 128] = np.triu(np.ones((128, 128)))
    consts[0:64, NDC * 192 + 128 :] = np.eye(64)
    consts = consts.astype(ml_dtypes.bfloat16)
    xT = np.ascontiguousarray(np.transpose(x, (0, 2, 1))).astype(ml_dtypes.bfloat16)

    out = np.empty((B, N, D_OUT), np.float32)
    kT_acc = [np.zeros((64, 0), ml_dtypes.bfloat16) for _ in range(B)]
    vx_acc = [np.zeros((128, 0), ml_dtypes.bfloat16) for _ in range(B)]
    exec_ns = []
    kw = {}
    if _trace:
        kw = dict(trace=True, trace_cores=[0], tmpdir=_tracedir)
    for i, nc in enumerate(progs):
        a, bb = SHARDS[i], SHARDS[i + 1]
        in_maps = []
        for bi in range(B):
            m = {
                "xT": np.ascontiguousarray(xT[bi, :, a:bb]),
                "consts": consts,
            }
            if a:
                m["kT_in"] = np.ascontiguousarray(kT_acc[bi])
                m["vx_in"] = np.ascontiguousarray(vx_acc[bi])
            in_maps.append(m)
        core_ids = [0, 1, 2, 3] if i % 2 == 0 else [4, 5, 6, 7]
        res = run_bass_kernel_spmd(nc, in_maps, core_ids=core_ids, **kw)
        exec_ns.append(res.exec_time_ns)
        for bi in range(B):
            o = np.asarray(res.results[bi]["o"], dtype=np.float32)
            out[bi, a:bb] = o[:, :64] / o[:, 64:65]
            kT_acc[bi] = np.concatenate(
                [kT_acc[bi], np.asarray(res.results[bi]["kT_out"])], axis=1
            )
            vx_acc[bi] = np.concatenate(
                [vx_acc[bi], np.asarray(res.results[bi]["vx_out"])], axis=1
            )
    _cache["last_exec_ns"] = tuple(exec_ns)
    return out
